# revision 2
# baseline (speedup 1.0000x reference)
"""Trainium2 Bass kernel for nn_ConvBlockFD — 1D-Winograd F(2,3) version.

y = relu(fdconv2(relu(fdconv1(x)))), fdconv = per-sample 3x3 conv with
attention-mixed kernel bank (see reference).

vs the direct-conv baseline: each 3x3 conv is computed as 3 row-taps of a
1D Winograd F(2,3) conv along W — 6 multiplies per output instead of 9,
a 1.5x reduction in PE work (the bottleneck engine). All storage fp16.

Per conv:
  V_u = 1D input transform (4 tensor add/subs per row band)
  M_u[co, r, j] = sum_{dy, ci} Wt[dy,u][ci,co] * V_u[ci, r+dy-1, j]
    (4 separate PSUM accumulators, 12/24 matmuls per (cog, band))
  y[2j]   = M_0 + M_1 + M_2
  y[2j+1] = M_1 - M_2 - M_3
  epilogue: ACT relu+bias (+ GAP accum for the layer-2 attention).

All images (x, y1, output) are stored as de-interleaved even/odd column
PLANES with one zero pad col per side: with xe[j]=x[2j], xo[j]=x[2j+1],
the four transform taps d0..d3 become CONTIGUOUS plane slices, so every
DVE/gpsimd op runs at full contiguous fp16 rate (strided stride-2 access
was the bottleneck of the naive layout). Host de/interleaves x and y.

Weight transforms (G-matrix side, with its /2) are linear -> folded into
the host-precomputed per-sample conv1 weights and the conv2 bank basis;
the device only mixes banks elementwise (attention weights from a GAP ->
MLP -> exp chain on device, softmax norm folded into the conv2 epilogue).
"""
import numpy as np

import concourse.bacc as bacc
import concourse.mybir as mybir
import concourse.tile as tile
from concourse.bass_utils import run_bass_kernel_spmd

F32 = mybir.dt.float32
F16 = mybir.dt.float16
AF = mybir.ActivationFunctionType
ALU = mybir.AluOpType
AX = mybir.AxisListType

N_CORES = 8
B, Cin, Cout, H, W = 16, 128, 256, 128, 128
S = B // N_CORES          # samples per core
K_NUM = 4
HW = H * W
P = 128
G2 = Cout // P            # channel groups = 2
XB = 16                   # row bands
XBR = H // XB             # output rows per band = 8
NJ = W // 2               # winograd tiles (cols per plane) = 64
NJ2 = NJ + 2              # padded plane width
NT = 12                   # transformed taps = 3 dy * 4 u
H2 = Cout // 4            # attention hidden = 64
CHUNKS = [(0,), (1,), (2,), (3,), (4, 5, 6, 7), (8, 9, 10, 11)]


def build_program():
    nc = bacc.Bacc("TRN2", target_bir_lowering=False, debug=False)

    # x/y in even-odd plane layout (planes padded with a zero col each side)
    x_d = nc.dram_tensor("x", [S, Cin, H, 2, NJ2], F16, kind="ExternalInput")
    wt1_d = nc.dram_tensor("wt1", [S, P, NT, Cout], F16, kind="ExternalInput")
    basis2_d = nc.dram_tensor("basis2", [P, K_NUM, NT, G2, Cout], F16,
                              kind="ExternalInput")
    a2w1_d = nc.dram_tensor("a2w1", [G2, P, H2], F32, kind="ExternalInput")
    a2b1_d = nc.dram_tensor("a2b1", [H2, 1], F32, kind="ExternalInput")
    a2w2_d = nc.dram_tensor("a2w2", [H2 + 1, K_NUM], F32, kind="ExternalInput")
    b1_d = nc.dram_tensor("b1", [G2, P, 1], F32, kind="ExternalInput")
    b2_d = nc.dram_tensor("b2", [G2, P, 1], F32, kind="ExternalInput")
    y_d = nc.dram_tensor("y", [S, G2, P, H, 2, NJ], F16, kind="ExternalOutput")

    with tile.TileContext(nc) as tc:
        with (
            tc.tile_pool(name="const", bufs=1) as cpool,
            tc.tile_pool(name="stage", bufs=2) as spool,
            tc.tile_pool(name="vband", bufs=3) as vpool,
            tc.tile_pool(name="outp", bufs=2) as opool,
            tc.tile_pool(name="psum", bufs=8, space="PSUM") as ppool,
        ):
            # ---- persistent SBUF ----
            # y1 planes: [cig, row(+2 ring), plane, col(+2 ring)]
            y1p = cpool.tile([P, G2, H + 2, 2, NJ2], F16, tag="y1p")
            wt1_t = [cpool.tile([P, NT, Cout], F16, tag=f"wt1_{s}", name=f"wt1_{s}")
                     for s in range(S)]
            basis2_t = cpool.tile([P, K_NUM, NT, G2, Cout], F16, tag="basis2")
            wd2_t = [cpool.tile([P, len(ch), G2, Cout], F16, tag=f"wd2_{c}",
                                name=f"wd2_{c}") for c, ch in enumerate(CHUNKS)]
            a2w1_t = [cpool.tile([P, H2], F32, tag=f"a2w1_{g}", name=f"a2w1_{g}")
                      for g in range(G2)]
            a2b1_t = cpool.tile([H2, 1], F32, tag="a2b1")
            a2w2_t = cpool.tile([H2 + 1, K_NUM], F32, tag="a2w2")
            b1_t = [cpool.tile([P, 1], F32, tag=f"b1_{g}", name=f"b1_{g}")
                    for g in range(G2)]
            b2_t = [cpool.tile([P, 1], F32, tag=f"b2_{g}", name=f"b2_{g}")
                    for g in range(G2)]
            gap_parts = cpool.tile([P, G2 * XB], F32, tag="gap_parts")
            gap_t = [cpool.tile([P, 1], F32, tag=f"gap_{g}", name=f"gap_{g}")
                     for g in range(G2)]
            h_aug = cpool.tile([H2 + 1, 1], F32, tag="h_aug")
            e_t = cpool.tile([1, K_NUM], F32, tag="e_t")
            sum_t = cpool.tile([1, 1], F32, tag="sum_t")
            rcp_t = cpool.tile([1, 1], F32, tag="rcp_t")
            rcp_bc = cpool.tile([P, 1], F32, tag="rcp_bc")
            ones_row = cpool.tile([1, P], F32, tag="ones_row")
            zeros_t = cpool.tile([P, XBR, 2, NJ], F32, tag="zeros_t")

            def ring_y1():
                for g in range(G2):
                    nc.gpsimd.memset(y1p[:, g, 0, :, :], 0.0)
                    nc.gpsimd.memset(y1p[:, g, H + 1, :, :], 0.0)
                    for pl in range(2):
                        nc.gpsimd.memset(y1p[:, g, :, pl, 0], 0.0)
                        nc.gpsimd.memset(y1p[:, g, :, pl, NJ2 - 1], 0.0)

            def xform(dst, e, o, vops, gops):
                """1D F(2,3) input transform from even/odd padded plane
                slices e, o (last dim NJ2) into dst planes [..., 4, R, NJ].
                vops/gops: which of the 4 taps run on vector/gpsimd."""
                def emit(u):
                    eng = nc.vector if u in vops else nc.gpsimd
                    if u == 0:
                        eng.tensor_sub(dst[0], o[..., 0:NJ], o[..., 1:NJ + 1])
                    elif u == 1:
                        eng.tensor_add(dst[1], e[..., 1:NJ + 1], o[..., 1:NJ + 1])
                    elif u == 2:
                        eng.tensor_sub(dst[2], o[..., 1:NJ + 1], e[..., 1:NJ + 1])
                    else:
                        eng.tensor_sub(dst[3], e[..., 1:NJ + 1], e[..., 2:NJ2])
                for u in range(4):
                    emit(u)

            def load_band(s, b, split=False):
                """DMA x plane rows for band b + transform into a v1 band."""
                g0 = XBR * b
                r_lo = max(g0 - 1, 0)
                r_hi = min(g0 + XBR, H - 1)
                n = r_hi - r_lo + 1
                l_lo = (r_lo + 1) - g0
                st = spool.tile([P, XBR + 2, 2, NJ2], F16, tag="xstage",
                                name="xstage")
                if b == 0:
                    nc.gpsimd.memset(st[:, 0, :, :], 0.0)
                if b == XB - 1:
                    nc.gpsimd.memset(st[:, XBR + 1, :, :], 0.0)
                if split:
                    h1 = n // 2
                    nc.sync.dma_start(st[:, l_lo:l_lo + h1], x_d[s, :, r_lo:r_lo + h1])
                    nc.scalar.dma_start(st[:, l_lo + h1:l_lo + n],
                                        x_d[s, :, r_lo + h1:r_hi + 1])
                else:
                    eng = nc.sync if b % 2 == 0 else nc.gpsimd
                    eng.dma_start(st[:, l_lo:l_lo + n], x_d[s, :, r_lo:r_hi + 1])
                v1b = vpool.tile([P, 4, XBR + 2, NJ], F16, tag="v1b", name="v1b")
                xform([v1b[:, u] for u in range(4)],
                      st[:, :, 0], st[:, :, 1], vops=(1, 2, 3), gops=(0,))
                return v1b

            def inverse(ps, asm):
                """4 u-PSUMs [P, XBR, NJ] -> asm planes [P, XBR, 2, NJ].
                M1/M2 staged via ACT so 2 of 4 DVE ops are pure fp16."""
                c1 = opool.tile([P, XBR, NJ], F16, tag="c1", name="c1")
                c2 = opool.tile([P, XBR, NJ], F16, tag="c2", name="c2")
                t0 = opool.tile([P, XBR, NJ], F16, tag="t0", name="t0")
                t1 = opool.tile([P, XBR, NJ], F16, tag="t1", name="t1")
                nc.scalar.activation(c1[:, :, :], ps[1][:, :, :], AF.Copy)
                nc.scalar.activation(c2[:, :, :], ps[2][:, :, :], AF.Copy)
                nc.vector.tensor_add(t0[:, :, :], ps[0][:, :, :], c1[:, :, :])
                nc.vector.tensor_add(asm[:, :, 0, :], t0[:, :, :], c2[:, :, :])
                nc.vector.tensor_sub(t1[:, :, :], c1[:, :, :], c2[:, :, :])
                nc.vector.tensor_sub(asm[:, :, 1, :], t1[:, :, :], ps[3][:, :, :])

            # ---- startup ----
            v1_first = load_band(0, 0, split=True)
            nc.sync.dma_start(wt1_t[0][:], wt1_d[0])
            for g in range(G2):
                nc.sync.dma_start(b1_t[g][:], b1_d[g])
            nc.gpsimd.memset(h_aug[H2:H2 + 1, 0:1], 1.0)
            nc.gpsimd.memset(ones_row[0:1, :], 1.0)
            nc.gpsimd.memset(zeros_t[:, :, :, :], 0.0)
            ring_y1()

            def partial_mlp(g, h_ps):
                nc.vector.tensor_reduce(
                    gap_t[g][:, 0:1], gap_parts[:, g * XB:(g + 1) * XB],
                    AX.X, ALU.add)
                nc.tensor.matmul(h_ps[:, 0:1], a2w1_t[g][:, :], gap_t[g][:, 0:1],
                                 start=(g == 0), stop=(g == G2 - 1))

            for s in range(S):
                h_ps = ppool.tile([H2, 1], F32, tag="ps", name="h_ps")
                # ---- conv1 ----
                v1b = v1_first if s == 0 else load_band(s, 0)
                for b in range(XB):
                    cur = v1b
                    if b + 1 < XB:
                        v1b = load_band(s, b + 1)
                    r0 = b * XBR
                    for cog in range(G2):
                        ps = [ppool.tile([P, XBR, NJ], F32, tag="ps", name=f"ps{u}")
                              for u in range(4)]
                        for dy in range(3):
                            for u in range(4):
                                nc.tensor.matmul(
                                    ps[u][:, :, :],
                                    wt1_t[s][:, dy * 4 + u, cog * P:(cog + 1) * P],
                                    cur[:, u, dy:dy + XBR, :],
                                    start=(dy == 0), stop=(dy == 2))
                        asm = opool.tile([P, XBR, 2, NJ], F16, tag="asm", name="asm")
                        inverse(ps, asm)
                        col = cog * XB + b
                        dst = y1p[:, cog, r0 + 1:r0 + 1 + XBR, :, 1:NJ + 1]
                        if b == XB - 1 and cog == G2 - 1:
                            # final tile epilogue on DVE: the attention chain
                            # isn't gated on the (deep) ACT queue
                            nc.vector.scalar_tensor_tensor(
                                dst, asm[:, :, :, :], b1_t[cog][:, 0:1],
                                zeros_t[:, :, :, :], ALU.add, ALU.max,
                                accum_out=gap_parts[:, col:col + 1])
                        else:
                            nc.scalar.activation(
                                dst, asm[:, :, :, :], AF.Relu,
                                bias=b1_t[cog][:, 0:1],
                                accum_out=gap_parts[:, col:col + 1])
                    if s == 0 and b == 0:
                        nc.sync.dma_start(wt1_t[1][:], wt1_d[1])
                        for g in range(G2):
                            nc.sync.dma_start(a2w1_t[g][:], a2w1_d[g])
                            nc.sync.dma_start(b2_t[g][:], b2_d[g])
                        nc.sync.dma_start(a2b1_t[:], a2b1_d[:])
                        nc.sync.dma_start(a2w2_t[:], a2w2_d[:])
                        nc.sync.dma_start(basis2_t[:], basis2_d[:])

                # ---- layer-2 attention ----
                partial_mlp(0, h_ps)
                partial_mlp(1, h_ps)
                nc.scalar.activation(h_aug[:H2, 0:1], h_ps[:, 0:1], AF.Relu,
                                     bias=a2b1_t[:, 0:1])
                l_ps = ppool.tile([1, K_NUM], F32, tag="ps", name="l_ps")
                nc.tensor.matmul(l_ps[0:1, :], h_aug[:, 0:1], a2w2_t[:, :],
                                 start=True, stop=True)
                nc.scalar.activation(e_t[0:1, :], l_ps[0:1, :], AF.Exp,
                                     accum_out=sum_t[0:1, 0:1])
                e_bc = ppool.tile([P, K_NUM], F32, tag="ps", name="e_bc")
                nc.tensor.matmul(e_bc[:, :], ones_row[0:1, :], e_t[0:1, :],
                                 start=True, stop=True)
                nc.vector.reciprocal(rcp_t[0:1, 0:1], sum_t[0:1, 0:1])
                nc.gpsimd.partition_broadcast(rcp_bc[:, 0:1], rcp_t[0:1, 0:1])
                # ---- mix wd2 chunks (unnormalized attention weights) ----
                for c, ch in enumerate(CHUNKS):
                    t0, t1 = ch[0], ch[-1] + 1
                    nc.vector.scalar_tensor_tensor(
                        wd2_t[c][:, :, :, :], basis2_t[:, 0, t0:t1, :, :],
                        e_bc[:, 0:1], basis2_t[:, 0, t0:t1, :, :],
                        ALU.mult, ALU.bypass)
                    for k in range(1, K_NUM):
                        nc.vector.scalar_tensor_tensor(
                            wd2_t[c][:, :, :, :], basis2_t[:, k, t0:t1, :, :],
                            e_bc[:, k:k + 1], wd2_t[c][:, :, :, :],
                            ALU.mult, ALU.add)

                # ---- conv2 ----
                TAP2CHUNK = {t: (c, i) for c, ch in enumerate(CHUNKS)
                             for i, t in enumerate(ch)}

                def lhsT2(t, cig, cog):
                    c, i = TAP2CHUNK[t]
                    return wd2_t[c][:, i, cig, cog * P:(cog + 1) * P]

                for b in range(XB):
                    r0 = b * XBR
                    # V2 bands for BOTH cigs in single wide ops
                    v2b = spool.tile([P, 4, G2, XBR + 2, NJ], F16,
                                     tag="v2b", name="v2b")
                    xform([v2b[:, u] for u in range(4)],
                          y1p[:, :, r0:r0 + XBR + 2, 0],
                          y1p[:, :, r0:r0 + XBR + 2, 1],
                          vops=(1, 2, 3), gops=(0,))
                    if s + 1 < S and b == XB - 2:
                        v1_next = load_band(s + 1, 0)
                    for cog in range(G2):
                        ps = [ppool.tile([P, XBR, NJ], F32, tag="ps", name=f"q{u}")
                              for u in range(4)]
                        for t in range(NT):
                            dy, u = t // 4, t % 4
                            for cig in range(G2):
                                nc.tensor.matmul(
                                    ps[u][:, :, :], lhsT2(t, cig, cog),
                                    v2b[:, u, cig, dy:dy + XBR, :],
                                    start=(dy == 0 and cig == 0),
                                    stop=(dy == 2 and cig == G2 - 1))
                        asm = opool.tile([P, XBR, 2, NJ], F16, tag="asm", name="asm")
                        inverse(ps, asm)
                        o = opool.tile([P, XBR, 2, NJ], F16, tag="o", name="o")
                        nc.scalar.activation(o[:, :, :, :], asm[:, :, :, :],
                                             AF.Relu, bias=b2_t[cog][:, 0:1],
                                             scale=rcp_bc[:, 0:1])
                        nc.sync.dma_start(y_d[s, cog, :, r0:r0 + XBR], o[:, :, :, :])
                if s + 1 < S:
                    v1_first = v1_next

    nc.compile()
    return nc


_nc_cache = None


def _get_nc():
    global _nc_cache
    if _nc_cache is None:
        _nc_cache = build_program()
    return _nc_cache


def _irfft_basis(w_fr, w_fi):
    return np.fft.irfft2(w_fr + 1j * w_fi, s=(3, 3), axes=(-2, -1)).astype(np.float32)


def _softmax(v):
    e = np.exp(v - v.max(axis=-1, keepdims=True))
    return e / e.sum(axis=-1, keepdims=True)


def _wg_weights(w):
    """[..., 3(dx), Co] -> [..., 4(u), Co] 1D F(2,3) weight transform."""
    w0, w1, w2 = w[..., 0, :], w[..., 1, :], w[..., 2, :]
    return np.stack([w0, (w0 + w1 + w2) * 0.5, (w0 - w1 + w2) * 0.5, w2], axis=-2)


def prepare_inputs(inputs):
    xf = np.asarray(inputs['x'], dtype=np.float32)
    # even/odd plane layout with zero pad col each side
    xpl = np.zeros((B, Cin, H, 2, NJ2), np.float16)
    xpl[..., 0, 1:NJ + 1] = xf[..., 0::2]
    xpl[..., 1, 1:NJ + 1] = xf[..., 1::2]

    w1 = _irfft_basis(np.asarray(inputs['w1_fr']), np.asarray(inputs['w1_fi']))
    w2 = _irfft_basis(np.asarray(inputs['w2_fr']), np.asarray(inputs['w2_fi']))
    gap = xf.mean((2, 3))
    h = np.maximum(gap @ np.asarray(inputs['a1w1']) + np.asarray(inputs['a1b1']), 0)
    attn1 = _softmax(h @ np.asarray(inputs['a1w2']) + np.asarray(inputs['a1b2']))
    w1T = w1.transpose(0, 2, 3, 4, 1)                     # [K, Ci, dy, dx, Co]
    wd1 = np.einsum('bk,kiyxc->biyxc', attn1, w1T)        # [B, Ci, 3, 3, Co]
    wt1 = _wg_weights(wd1).reshape(B, Cin, NT, Cout).astype(np.float16)

    w2T = w2.transpose(0, 2, 3, 4, 1)                     # [K, Ci2, 3, 3, Co]
    w2t = _wg_weights(w2T).reshape(K_NUM, Cout, NT, Cout)
    basis2 = np.ascontiguousarray(
        w2t.reshape(K_NUM, G2, P, NT, Cout).transpose(2, 0, 3, 1, 4)
    ).astype(np.float16)

    a2w1 = (np.asarray(inputs['a2w1'], dtype=np.float32) / HW).reshape(G2, P, H2)
    a2b1 = np.asarray(inputs['a2b1'], dtype=np.float32).reshape(-1, 1)
    a2w2 = np.ascontiguousarray(np.vstack([
        np.asarray(inputs['a2w2'], dtype=np.float32),
        np.asarray(inputs['a2b2'], dtype=np.float32).reshape(1, -1)]))
    b1 = np.asarray(inputs['b1'], dtype=np.float32).reshape(G2, P, 1)
    b2 = np.asarray(inputs['b2'], dtype=np.float32).reshape(G2, P, 1)

    in_maps = []
    for c in range(N_CORES):
        sl = slice(c * S, (c + 1) * S)
        in_maps.append({
            'x': np.ascontiguousarray(xpl[sl]),
            'wt1': np.ascontiguousarray(wt1[sl]),
            'basis2': basis2,
            'a2w1': a2w1, 'a2b1': a2b1, 'a2w2': a2w2,
            'b1': b1, 'b2': b2,
        })
    return in_maps


def run(inputs, trace=False, **kwargs):
    nc = _get_nc()
    in_maps = prepare_inputs(inputs)
    res = run_bass_kernel_spmd(nc, in_maps, list(range(N_CORES)),
                               trace=trace, **kwargs)
    y = np.empty((B, Cout, H, W), np.float32)
    for c, r in enumerate(res.results):
        yp = r['y'].reshape(S, Cout, H, 2, NJ).astype(np.float32)
        y[c * S:(c + 1) * S, ..., 0::2] = yp[..., 0, :]
        y[c * S:(c + 1) * S, ..., 1::2] = yp[..., 1, :]
    return y, res


def kernel(**inputs) -> np.ndarray:
    y, _ = run(inputs, trace=False)
    return y
